# revision 2
# baseline (speedup 1.0000x reference)
"""Contrastive-loss kernel for trn2 (8 NeuronCores, SPMD).

The reference loss reduces to a Gram matrix G = F.T @ F over the
flattened input F [N=524288, T=64] (128 MiB fp32), followed by a tiny
[64,64] masked margin reduction.  Each core streams a contiguous
row-shard of F (16 MiB) through SBUF in 2 MiB tiles, accumulating
chunk.T @ chunk matmuls (K=128, M=N=64) into one PSUM accumulator.
The 8 partial [64,64] Grams are summed on the host, where the masked
margin reduction (negligible work) also runs.
"""

import numpy as np

import concourse.bacc as bacc
import concourse.mybir as mybir
from concourse import tile
from concourse.bass_utils import run_bass_kernel_spmd

MARGIN = 60000.0
S = 64                      # time steps (Gram dim)
N_TOTAL = 2 * 8 * 32 * 32 * 32   # 524288 flattened rows
N_CORES = 8
N_SHARD = N_TOTAL // N_CORES     # 65536 rows per core
P = 128                     # SBUF partitions
ROWS_PER_TILE = 8192        # rows per DMA tile -> [128, 4096] f32 = 2 MiB
RPP = ROWS_PER_TILE // P    # rows per partition within a tile (64)
FREE = RPP * S              # free dim of an input tile (4096)
N_TILES = N_SHARD // ROWS_PER_TILE   # 8 DMA tiles per core
IN_BUFS = 3

_CACHE = {}
LAST_RESULTS = None         # BassKernelResults of the most recent run


def _build_nc():
    nc = bacc.Bacc("TRN2", target_bir_lowering=False, debug=False,
                   num_devices=N_CORES)
    x = nc.dram_tensor("x", [N_SHARD, S], mybir.dt.float32,
                       kind="ExternalInput")
    g = nc.dram_tensor("g", [S, S], mybir.dt.float32, kind="ExternalOutput")
    xv = x[:].rearrange("(n p r) c -> n p (r c)", p=P, r=RPP)

    with tile.TileContext(nc) as tc:
        with (
            tc.tile_pool(name="xin", bufs=IN_BUFS) as pool,
            tc.tile_pool(name="acc", bufs=1, space="PSUM") as psum_pool,
            tc.tile_pool(name="out", bufs=1) as out_pool,
        ):
            acc = psum_pool.tile([S, S], mybir.dt.float32)
            for i in range(N_TILES):
                # SWDGE casts fp32 -> bf16 inline during the HBM->SBUF DMA;
                # bf16 matmul runs the PE at full rate (fp32 is half rate).
                t = pool.tile([P, FREE], mybir.dt.bfloat16)
                nc.gpsimd.dma_start(t[:], xv[i])
                for j in range(FREE // S):
                    c = t[:, j * S:(j + 1) * S]
                    nc.tensor.matmul(
                        acc[:], c, c,
                        start=(i == 0 and j == 0),
                        stop=(i == N_TILES - 1 and j == FREE // S - 1),
                    )
            o = out_pool.tile([S, S], mybir.dt.float32)
            nc.vector.tensor_copy(o[:], acc[:])
            nc.sync.dma_start(g[:], o[:])

    nc.compile()
    return nc


def get_nc():
    if "nc" not in _CACHE:
        _CACHE["nc"] = _build_nc()
    return _CACHE["nc"]


def _device_partial_grams(flat: np.ndarray, **run_kwargs) -> np.ndarray:
    """Run the SPMD bass kernel; return the 8 partial Grams [8, 64, 64]."""
    global LAST_RESULTS
    nc = get_nc()
    in_maps = [
        {"x": flat[c * N_SHARD:(c + 1) * N_SHARD]} for c in range(N_CORES)
    ]
    LAST_RESULTS = run_bass_kernel_spmd(
        nc, in_maps, core_ids=list(range(N_CORES)), **run_kwargs
    )
    return np.stack([LAST_RESULTS.results[c]["g"] for c in range(N_CORES)])


def kernel(input: np.ndarray, **run_kwargs) -> np.ndarray:
    flat = np.ascontiguousarray(
        np.asarray(input, dtype=np.float32).reshape(N_TOTAL, S)
    )
    partials = _device_partial_grams(flat, **run_kwargs)

    gram = partials.astype(np.float64).sum(axis=0)
    sq = np.diag(gram)
    dist = sq[:, None] + sq[None, :] - 2.0 * gram
    idx = np.arange(S)
    lower = idx[:, None] > idx[None, :]
    adjacent = (idx[:, None] - idx[None, :]) == 1
    per_pair = np.where(adjacent, np.maximum(0.0, MARGIN - dist), dist)
    loss = np.where(lower, per_pair, 0.0).sum() / (S * (S - 1) * 1000)
    return np.asarray(loss, dtype=np.float32)
